# revision 4
# baseline (speedup 1.0000x reference)
"""CatLayer Trainium2 kernel (fp16 datapath, fp32 PSUM accumulate).

Math: out[i,j,b,:] = W @ leaky_relu(concat(x[i,b,:], x[j,b,:])) + bias
Since leaky_relu is elementwise over the concat:
    y  = leaky_relu(x)                    # (l, b, d)
    A  = y @ W[:, :d].T + bias            # (l, b, d)   "xi half"
    B  = y @ W[:, d:].T                   # (l, b, d)   "xj half"
    out[i,j,b,:] = A[i,b,:] + B[j,b,:]

Sharding: i-rows of the (l x l) pair grid over 8 cores (12 rows each).
Every core computes B for all j from full x; A from its own 12 i-rows.

The whole on-device datapath is fp16 (the harness gate is rel<2e-2;
fp16 end-to-end costs ~1e-3): halves every DMA byte, gives 1 cyc/row
matmuls on PE and the packed-2-byte DVE mode for the adds. PSUM
accumulation stays fp32.

Per-core input packing (host side, all fp16, partition dim leading):
    xT   (128, KT*T): xT[p, k*T + t] = x_rot[t, 128k+p] where x_rot is
         x.reshape(T, D) cyclically rotated by -r*TI rows, so each
         core's own 192 (i,b) rows sit at t in [0, TI). The A-path
         reads them as a column slice of yT; no separate xi input.
         Host un-rotates the output with np.roll after the gather.
    W_in (128, 8*D): W_in[p, g*D+c] = W.T[128g+p, c] (g<4: W1, g>=4: W2)
    bias (1, D)
    out  (12*l*b, d) fp16, host converts to fp32.

Schedule (the cost model serializes all DMA transfers on one device at
360 B/ns; ~650 ns SEQ+HWDGE per DMA on the HWDGE queues; SWDGE gen on
the Pool ENGINE ~1us per DMA but cheap on Pool SEQ):
    SP  : xc = x columns [0,256) of each k slice (covers the A rows and
          j-tiles 0..1), then even-il output stores
    Pool: W1, W2, xr = x columns [256,1536), ec, bias (the serial SWDGE
          gen enforces exactly this DMA order), then odd-il stores
    ACT : Prelu slices + B-path PSUM drains + late abc drains
    PE  : A/B matmuls, one-hot E-matmul broadcast of A[i], warm-up
    DVE : tensor_add for all output tiles + A drains + early abc drains
"""

import numpy as np
from contextlib import ExitStack

import concourse.bacc as bacc
import concourse.mybir as mybir
from concourse import tile
from concourse.bass_utils import run_bass_kernel_spmd

F32 = mybir.dt.float32
F16 = mybir.dt.float16
AF = mybir.ActivationFunctionType

L, Bdim, D = 96, 16, 512
NCORES = 8
LPC = L // NCORES          # 12 i-rows per core
T = L * Bdim               # 1536 (j,b) rows
NT = T // 128              # 12 j-tiles
KT = D // 128              # 4 k-tiles
TI = LPC * Bdim            # 192 own (i,b) rows
NEG_SLOPE = 0.1
XC = 256                   # first x chunk: own A rows + j-tiles 0,1


def build_nc(repeats: int = 1, group_sizes=(1, 1, 2, 4, 4), warm=16):
    """Build the per-core Bass program (identical on all cores)."""
    assert sum(group_sizes) == NT
    g_off = [0]
    for g in group_sizes:
        g_off.append(g_off[-1] + g)

    nc = bacc.Bacc("TRN2", target_bir_lowering=False, debug=False)

    xT = nc.dram_tensor("xT", (128, KT * T), F16, kind="ExternalInput")
    w_in = nc.dram_tensor("w_in", (128, 2 * KT * D), F16, kind="ExternalInput")
    bias = nc.dram_tensor("bias", (1, D), F16, kind="ExternalInput")
    out = nc.dram_tensor("out", (LPC * T, D), F16, kind="ExternalOutput")

    # One-hot E for the 16->128 partition broadcast of A rows, replicated
    # with period 32 down all 128 partitions so any legal 32-aligned window
    # has identical content: ec[g, a*128 + p] == 1 iff g % 32 == 16*a + p % 16
    # (g = partition = contraction row, p = output partition, a = half).
    ec_np = np.zeros((128, 256), np.float16)
    for g in range(128):
        for a in range(2):
            for p in range(128):
                if g % 32 == 16 * a + (p % 16):
                    ec_np[g, a * 128 + p] = 1.0
    ec_dram = nc.inline_tensor(ec_np, "Ec")

    with tile.TileContext(nc) as tc, ExitStack() as ctx:
        persist = ctx.enter_context(tc.tile_pool(name="persist", bufs=1))
        stage = ctx.enter_context(tc.tile_pool(name="stage", bufs=2))
        psum = ctx.enter_context(tc.tile_pool(name="psum", bufs=6, space="PSUM"))
        outp = ctx.enter_context(tc.tile_pool(name="outp", bufs=3))

        ones_sb = persist.tile([1, 128], F16, tag="ones", name="ones_sb")
        nc.vector.memset(ones_sb[:], 1.0)

        # ---- x chunk 1 on SP: the critical-path load
        x_st = stage.tile([128, KT * T], F16, tag="x_st", bufs=1, name="x_st")
        x_v = x_st[:].rearrange("p (k t) -> p k t", k=KT)
        xT_v = xT.ap().rearrange("p (k t) -> p k t", k=KT)
        nc.sync.dma_start(x_v[:, :, :XC], xT_v[:, :, :XC])

        # ---- W halves + x rest + constants on Pool; the Pool ENGINE
        # serializes SWDGE descriptor generation, which pins the DMA
        # order to exactly this sequence.
        w_sb = persist.tile([128, 2 * KT * D], F16, tag="w", name="w_sb")
        nc.gpsimd.dma_start(w_sb[:, : KT * D], w_in[:, : KT * D])        # W1
        nc.gpsimd.dma_start(w_sb[:, KT * D :], w_in[:, KT * D :])        # W2
        nc.gpsimd.dma_start(x_v[:, :, XC:], xT_v[:, :, XC:])             # xr
        ec_all = persist.tile([128, 256], F16, tag="ec", name="ec_all")
        nc.gpsimd.dma_start(ec_all[:], ec_dram.ap())
        bias_sb = persist.tile([1, D], F16, tag="bias", name="bias_sb")
        nc.gpsimd.dma_start(bias_sb[:], bias[:])
        ec_sb = [ec_all[:, :128], ec_all[:, 128:]]

        def w1s(k):
            return w_sb[:, k * D : (k + 1) * D]

        def w2s(k):
            return w_sb[:, (KT + k) * D : (KT + k + 1) * D]

        # ---- PE warm-up: HAM ramps the PE clock only while it's busy.
        # Wide dummy matmuls keep it continuously busy (and fully ramped)
        # until the W load lands and real matmuls start.
        warm_ps = psum.tile([128, D], F32, tag="eps", bufs=2, name="warm_ps")
        for _ in range(warm):
            nc.tensor.matmul(
                warm_ps[:], ones_sb[:1, :], ones_sb[:1, :1].broadcast_to((1, D)),
                start=True, stop=True,
            )

        # ---- leaky relu slices: [0, XC) now; the rest emitted later at
        # the point of first use so the in-order ACT stream stays frontier.
        yT = persist.tile([128, KT * T], F16, tag="yT", name="yT")
        y_v = yT[:].rearrange("p (k t) -> p k t", k=KT)

        def relu(c0, c1):
            nc.scalar.activation(
                y_v[:, :, c0:c1], x_v[:, :, c0:c1], AF.Prelu, alpha=NEG_SLOPE
            )

        relu(0, XC)

        # ---- A = leaky_relu(x_own) @ W1.T + bias in three M-groups
        # (128, 32, 64 rows) sliced straight out of yT columns [0, TI).
        # The 32-row group re-covers rows 96..127 so every E-matmul
        # window starts at a legal base partition ({0, 32, 64}).
        a_parts = {}   # w -> (tile, offset)

        def emit_a(tag, rows, col0, windows):
            aps = psum.tile(
                [rows, D], F32, tag="ps32", bufs=2,
                padded_shape=[128, D], name=f"aps_{tag}"
            )
            for k in range(KT):
                nc.tensor.matmul(
                    aps[:],
                    yT[:, k * T + col0 : k * T + col0 + rows],
                    w1s(k),
                    start=(k == 0),
                    stop=False,
                )
            nc.tensor.matmul(
                aps[:], ones_sb[:1, :rows], bias_sb[:1, :],
                start=False, stop=True,
            )
            aw = persist.tile([rows, D], F16, tag=f"a_{tag}", name=f"a_{tag}")
            nc.vector.tensor_copy(aw[:], aps[:])
            for w, off in windows:
                a_parts[w] = (aw, off)

        out_v = out.rearrange("(i j p) c -> i p j c", i=LPC, p=128)
        abc = persist.tile([128, LPC * D], F16, tag="abc", name="abc")
        n_grp = len(group_sizes)
        b_grp = [None] * n_grp

        def emit_bgroup(g):
            gsz = group_sizes[g]
            bg = persist.tile(
                [128, gsz * D], F16, tag=f"b_grp{g}", name=f"b_grp{g}"
            )
            for q in range(gsz):
                jt = g_off[g] + q
                bps = psum.tile(
                    [128, D], F32, tag="ps", bufs=4, name=f"bps_{jt}"
                )
                for k in range(KT):
                    nc.tensor.matmul(
                        bps[:],
                        yT[:, k * T + 128 * jt : k * T + 128 * (jt + 1)],
                        w2s(k),
                        start=(k == 0),
                        stop=(k == KT - 1),
                    )
                nc.scalar.activation(bg[:, q * D : (q + 1) * D], bps[:], AF.Copy)
            b_grp[g] = bg

        def emit_abc(il):
            w, par = divmod(il, 2)
            src, off = a_parts[w]
            eps = psum.tile([128, D], F32, tag="eps", bufs=2, name=f"eps_{il}")
            nc.tensor.matmul(
                eps[:],
                ec_sb[par][off : off + 32],
                src[off : off + 32, :],
                start=True,
                stop=True,
            )
            # early slices drain on DVE (ACT is stuck behind the relus in
            # its in-order stream); later ones go to ACT which has slack.
            if il < 4:
                nc.vector.tensor_copy(abc[:, il * D : (il + 1) * D], eps[:])
            else:
                nc.scalar.activation(
                    abc[:, il * D : (il + 1) * D], eps[:], AF.Copy
                )

        def emit_add(il, g):
            gsz = group_sizes[g]
            ot = outp.tile(
                [128, gsz * D], F16, tag="ot", bufs=10, name=f"ot_{il}_{g}"
            )
            a_slice = abc[:, il * D : (il + 1) * D]
            nc.vector.tensor_add(
                ot[:].rearrange("p (j c) -> p j c", c=D),
                b_grp[g][:].rearrange("p (j c) -> p j c", c=D),
                a_slice.unsqueeze(1).broadcast_to((128, gsz, D)),
            )
            # stores parity-split over the SP (HWDGE) and Pool (SWDGE)
            # queues so per-DMA issue overhead never gates the transfer
            # stream.
            q = nc.sync if il % 2 == 0 else nc.gpsimd
            q.dma_start(
                out_v[il, :, g_off[g] : g_off[g + 1], :],
                ot[:].rearrange("p (j c) -> p j c", c=D),
            )

        ready_il = []
        ready_g = []

        def unlock_il(*ils):
            for il in ils:
                emit_abc(il)
            for il in ils:
                ready_il.append(il)
                for g in ready_g:
                    emit_add(il, g)

        def unlock_g(g):
            emit_bgroup(g)
            ready_g.append(g)
            for il in ready_il:
                emit_add(il, g)

        # windows: w0..2 live in the 128-row A group at offsets 0/32/64,
        # w3 in its own 32-row group, w4..5 in the 64-row group.
        emit_a("g0", 128, 0, [(0, 0), (1, 32), (2, 64)])
        unlock_il(0, 1)
        unlock_g(0)            # j0 (within xc)
        unlock_g(1)            # j1 (within xc)
        relu(XC, 512)
        unlock_il(2, 3)
        unlock_g(2)            # j2, j3
        relu(512, 1024)
        emit_a("g0b", 32, 96, [(3, 0)])
        unlock_il(4, 5)
        unlock_il(6, 7)
        unlock_g(3)            # j4..j7
        relu(1024, T)
        emit_a("g1", 64, 128, [(4, 0), (5, 32)])
        unlock_il(8, 9)
        unlock_il(10, 11)
        unlock_g(4)            # j8..j11

    nc.compile()
    return nc


def _pack_kt(arr_t, nfree):
    """(D, nfree) k-major -> (128, KT*nfree) partition-packed SBUF layout."""
    return np.ascontiguousarray(
        arr_t.reshape(KT, 128, nfree).transpose(1, 0, 2).reshape(128, KT * nfree)
    )


def make_in_maps(x, W, bias):
    x = np.asarray(x, np.float16)
    W = np.asarray(W, np.float16)
    bias = np.asarray(bias, np.float16)
    x_flat = x.reshape(T, D)
    w_all = np.ascontiguousarray(
        np.ascontiguousarray(W.T)
        .reshape(2 * KT, 128, D)
        .transpose(1, 0, 2)
        .reshape(128, 2 * KT * D)
    )
    b2 = np.ascontiguousarray(bias.reshape(1, D))
    maps = []
    for r in range(NCORES):
        xr = np.roll(x_flat, -r * TI, axis=0)
        xTr = _pack_kt(np.ascontiguousarray(xr.T), T)
        maps.append({"xT": xTr, "w_in": w_all, "bias": b2})
    return maps


_NC_CACHE = {}


def get_nc(repeats=1, group_sizes=(1, 1, 2, 4, 4), warm=16):
    key = (repeats, tuple(group_sizes), warm)
    if key not in _NC_CACHE:
        _NC_CACHE[key] = build_nc(
            repeats=repeats, group_sizes=group_sizes, warm=warm
        )
    return _NC_CACHE[key]


def kernel(x, W, bias, group_sizes=(1, 1, 2, 4, 4), warm=16):
    nc = get_nc(1, group_sizes, warm)
    maps = make_in_maps(x, W, bias)
    res = run_bass_kernel_spmd(nc, maps, list(range(NCORES)))
    parts = []
    for r in range(NCORES):
        o = res.results[r]["out"].reshape(LPC, T, D)
        parts.append(np.roll(o, r * TI, axis=1))
    full = np.concatenate(parts, axis=0)          # (L, T, D)
    return full.reshape(L * L, Bdim, D).astype(np.float32)


# revision 14
# speedup vs baseline: 1.0822x; 1.0822x over previous
"""CatLayer Trainium2 kernel (fp16 datapath, fp32 PSUM accumulate).

Math: out[i,j,b,:] = W @ leaky_relu(concat(x[i,b,:], x[j,b,:])) + bias
Since leaky_relu is elementwise over the concat:
    y  = leaky_relu(x)                    # (l, b, d)
    A  = y @ W[:, :d].T + bias            # (l, b, d)   "xi half"
    B  = y @ W[:, d:].T                   # (l, b, d)   "xj half"
    out[i,j,b,:] = A[i,b,:] + B[j,b,:]

Sharding: i-rows of the (l x l) pair grid over 8 cores (12 rows each).
Every core computes B for all j from full x; A from its own 12 i-rows.

The whole on-device datapath is fp16 (the harness gate is rel<2e-2;
fp16 end-to-end costs ~1e-3): halves every DMA byte, gives 1 cyc/row
matmuls on PE and the packed-2-byte DVE mode for the adds. PSUM
accumulation stays fp32.

Per-core input packing (host side, all fp16, partition dim leading):
    xT   (128, KT*T): xT[p, k*T + t] = x_rot[t, 128k+p] where x_rot is
         x.reshape(T, D) cyclically rotated by -r*TI rows, so each
         core's own 192 (i,b) rows sit at t in [0, TI). The A-path
         reads them as a column slice of yT; no separate xi input.
         Host un-rotates the output with np.roll after the gather.
    W_in (128, 8*D): W_in[p, g*D+c] = W.T[128g+p, c] (g<4: W1, g>=4: W2)
    bias (1, D)
    out  (12*l*b, d) fp16, host converts to fp32.

Schedule (the cost model serializes all DMA transfers on one device at
360 B/ns; ~650 ns SEQ+HWDGE per DMA on the HWDGE queues; SWDGE gen on
the Pool ENGINE ~1us per DMA but cheap on Pool SEQ):
    SP  : xc = x columns [0,256) of each k slice (covers the A rows and
          j-tiles 0..1), then even-il output stores
    Pool: W1, W2, xr = x columns [256,1536), ec, bias (the serial SWDGE
          gen enforces exactly this DMA order), then odd-il stores
    ACT : Prelu slices + B-path PSUM drains + late abc drains
    PE  : A/B matmuls, one-hot E-matmul broadcast of A[i], warm-up
    DVE : tensor_add for all output tiles + A drains + early abc drains
"""

import numpy as np
from contextlib import ExitStack

import concourse.bacc as bacc
import concourse.mybir as mybir
from concourse import tile
from concourse.bass_utils import run_bass_kernel_spmd

F32 = mybir.dt.float32
F16 = mybir.dt.float16
AF = mybir.ActivationFunctionType

L, Bdim, D = 96, 16, 512
NCORES = 8
LPC = L // NCORES          # 12 i-rows per core
T = L * Bdim               # 1536 (j,b) rows
NT = T // 128              # 12 j-tiles
KT = D // 128              # 4 k-tiles
TI = LPC * Bdim            # 192 own (i,b) rows
NEG_SLOPE = 0.1
XC = 256                   # first x chunk: own A rows + j-tiles 0,1


def build_nc(
    repeats: int = 1,
    group_sizes=(1, 1, 2, 4, 4),
    warm=4,
    wfree=D,
    abc_dve=(0, 1),
    relu_split=((512, 768), (768, 1024), (1024, 1280), (1280, 1536)),
):
    """Build the per-core Bass program (identical on all cores)."""
    assert sum(group_sizes) == NT
    g_off = [0]
    for g in group_sizes:
        g_off.append(g_off[-1] + g)

    nc = bacc.Bacc("TRN2", target_bir_lowering=False, debug=False)

    xT = nc.dram_tensor("xT", (128, KT * T), F16, kind="ExternalInput")
    w_in = nc.dram_tensor("w_in", (128, 2 * KT * D), F16, kind="ExternalInput")
    bias = nc.dram_tensor("bias", (1, D), F16, kind="ExternalInput")
    out = nc.dram_tensor("out", (LPC * T, D), F16, kind="ExternalOutput")

    # One-hot E for the 16->128 partition broadcast of A rows, replicated
    # with period 32 down all 128 partitions so any legal 32-aligned window
    # has identical content: ec[g, a*128 + p] == 1 iff g % 32 == 16*a + p % 16
    # (g = partition = contraction row, p = output partition, a = half).
    ec_np = np.zeros((128, 256), np.float16)
    for g in range(128):
        for a in range(2):
            for p in range(128):
                if g % 32 == 16 * a + (p % 16):
                    ec_np[g, a * 128 + p] = 1.0
    ec_dram = nc.inline_tensor(ec_np, "Ec")

    with tile.TileContext(nc) as tc, ExitStack() as ctx:
        persist = ctx.enter_context(tc.tile_pool(name="persist", bufs=1))
        stage = ctx.enter_context(tc.tile_pool(name="stage", bufs=2))
        psum = ctx.enter_context(tc.tile_pool(name="psum", bufs=6, space="PSUM"))
        outp = ctx.enter_context(tc.tile_pool(name="outp", bufs=3))

        ones_sb = persist.tile([1, 128], F16, tag="ones", name="ones_sb")
        nc.vector.memset(ones_sb[:], 1.0)

        # ---- x chunk 1 on SP: the critical-path load
        x_st = stage.tile([128, KT * T], F16, tag="x_st", bufs=1, name="x_st")
        x_v = x_st[:].rearrange("p (k t) -> p k t", k=KT)
        xT_v = xT.ap().rearrange("p (k t) -> p k t", k=KT)
        nc.sync.dma_start(x_v[:, :, :XC], xT_v[:, :, :XC])

        # ec + bias are tiny; issue them on SP behind xc so their
        # transfers slip between the big Pool loads without delaying the
        # Pool SWDGE generation chain (which orders W1, W2, xr).
        ec_all = persist.tile([128, 256], F16, tag="ec", name="ec_all")
        nc.sync.dma_start(ec_all[:], ec_dram.ap())
        bias_sb = persist.tile([1, D], F16, tag="bias", name="bias_sb")
        nc.sync.dma_start(bias_sb[:], bias[:])

        # ---- W halves + x rest + constants on Pool; the Pool ENGINE
        # serializes SWDGE descriptor generation, which pins the DMA
        # order to exactly this sequence.
        w_sb = persist.tile([128, 2 * KT * D], F16, tag="w", name="w_sb")
        nc.gpsimd.dma_start(w_sb[:, : KT * D], w_in[:, : KT * D])        # W1
        nc.gpsimd.dma_start(w_sb[:, KT * D :], w_in[:, KT * D :])        # W2
        nc.gpsimd.dma_start(x_v[:, :, XC:], xT_v[:, :, XC:])             # xr
        ec_sb = [ec_all[:, :128], ec_all[:, 128:]]

        def w1s(k):
            return w_sb[:, k * D : (k + 1) * D]

        def w2s(k):
            return w_sb[:, (KT + k) * D : (KT + k + 1) * D]

        # ---- PE warm-up: HAM ramps the PE clock only while it's busy.
        # Wide dummy matmuls keep it continuously busy (and fully ramped)
        # until the W load lands and real matmuls start.
        warm_ps = psum.tile([128, D], F32, tag="eps", bufs=2, name="warm_ps")
        for _ in range(warm):
            nc.tensor.matmul(
                warm_ps[:, :wfree],
                ones_sb[:1, :],
                ones_sb[:1, :1].broadcast_to((1, wfree)),
                start=True, stop=True,
            )

        # ---- leaky relu slices: [0, XC) now; the rest emitted later at
        # the point of first use so the in-order ACT stream stays frontier.
        yT = persist.tile([128, KT * T], F16, tag="yT", name="yT")
        y_v = yT[:].rearrange("p (k t) -> p k t", k=KT)

        def relu(c0, c1):
            nc.scalar.activation(
                y_v[:, :, c0:c1], x_v[:, :, c0:c1], AF.Prelu, alpha=NEG_SLOPE
            )

        relu(0, XC)

        # ---- A = leaky_relu(x_own) @ W1.T + bias in three M-groups
        # (128, 32, 64 rows) sliced straight out of yT columns [0, TI).
        # The 32-row group re-covers rows 96..127 so every E-matmul
        # window starts at a legal base partition ({0, 32, 64}).
        a_parts = {}   # w -> (tile, offset)

        def emit_a(tag, rows, col0, windows):
            aps = psum.tile(
                [rows, D], F32, tag="ps32", bufs=2,
                padded_shape=[128, D], name=f"aps_{tag}"
            )
            for k in range(KT):
                nc.tensor.matmul(
                    aps[:],
                    yT[:, k * T + col0 : k * T + col0 + rows],
                    w1s(k),
                    start=(k == 0),
                    stop=False,
                )
            nc.tensor.matmul(
                aps[:], ones_sb[:1, :rows], bias_sb[:1, :],
                start=False, stop=True,
            )
            aw = persist.tile([rows, D], F16, tag=f"a_{tag}", name=f"a_{tag}")
            nc.vector.tensor_copy(aw[:], aps[:])
            for w, off in windows:
                a_parts[w] = (aw, off)

        out_v = out.rearrange("(i j p) c -> i p j c", i=LPC, p=128)
        abc = persist.tile([128, LPC * D], F16, tag="abc", name="abc")
        n_grp = len(group_sizes)
        b_grp = [None] * n_grp

        def emit_bgroup(g):
            gsz = group_sizes[g]
            bg = persist.tile(
                [128, gsz * D], F16, tag=f"b_grp{g}", name=f"b_grp{g}"
            )
            for q in range(gsz):
                jt = g_off[g] + q
                bps = psum.tile(
                    [128, D], F32, tag="ps", bufs=4, name=f"bps_{jt}"
                )
                for k in range(KT):
                    nc.tensor.matmul(
                        bps[:],
                        yT[:, k * T + 128 * jt : k * T + 128 * (jt + 1)],
                        w2s(k),
                        start=(k == 0),
                        stop=(k == KT - 1),
                    )
                nc.scalar.activation(bg[:, q * D : (q + 1) * D], bps[:], AF.Copy)
            b_grp[g] = bg

        def emit_abc(il):
            w, par = divmod(il, 2)
            src, off = a_parts[w]
            eps = psum.tile([128, D], F32, tag="eps", bufs=2, name=f"eps_{il}")
            nc.tensor.matmul(
                eps[:],
                ec_sb[par][off : off + 32],
                src[off : off + 32, :],
                start=True,
                stop=True,
            )
            # early slices drain on DVE (ACT is stuck behind the relus in
            # its in-order stream); later ones go to ACT which has slack.
            if il in abc_dve:
                nc.vector.tensor_copy(abc[:, il * D : (il + 1) * D], eps[:])
            else:
                nc.scalar.activation(
                    abc[:, il * D : (il + 1) * D], eps[:], AF.Copy
                )

        def emit_add(il, g):
            gsz = group_sizes[g]
            ot = outp.tile(
                [128, gsz * D], F16, tag="ot", bufs=10, name=f"ot_{il}_{g}"
            )
            a_slice = abc[:, il * D : (il + 1) * D]
            nc.vector.tensor_add(
                ot[:].rearrange("p (j c) -> p j c", c=D),
                b_grp[g][:].rearrange("p (j c) -> p j c", c=D),
                a_slice.unsqueeze(1).broadcast_to((128, gsz, D)),
            )
            # stores parity-split over the SP (HWDGE) and Pool (SWDGE)
            # queues so per-DMA issue overhead never gates the transfer
            # stream.
            q = nc.sync if il % 2 == 0 else nc.gpsimd
            q.dma_start(
                out_v[il, :, g_off[g] : g_off[g + 1], :],
                ot[:].rearrange("p (j c) -> p j c", c=D),
            )

        ready_il = []
        ready_g = []

        def unlock_il(*ils):
            for il in ils:
                emit_abc(il)
            for il in ils:
                ready_il.append(il)
                for g in ready_g:
                    emit_add(il, g)

        def unlock_g(g):
            emit_bgroup(g)
            ready_g.append(g)
            for il in ready_il:
                emit_add(il, g)

        # windows: w0..2 live in the 128-row A group at offsets 0/32/64,
        # w3 in its own 32-row group, w4..5 in the 64-row group.
        covered = XC // 128    # j-tiles whose relu has been emitted
        gi = 0

        def unlock_ready_groups():
            nonlocal gi
            while gi < n_grp and g_off[gi + 1] <= covered:
                unlock_g(gi)
                gi += 1

        a_specs = {
            "g0": (128, 0, [(0, 0), (1, 32), (2, 64)]),
            "g0b": (32, 96, [(3, 0)]),
            "g1": (64, 128, [(4, 0), (5, 32)]),
        }
        # il-pair unlocks spread over the relu/B frontier: once `covered`
        # j-tiles of relu have been emitted, emit the listed actions.
        il_sched = [
            (6, [("a", "g0b"), ("il", 4, 5)]),
            (8, [("il", 6, 7)]),
            (10, [("a", "g1"), ("il", 8, 9)]),
            (12, [("il", 10, 11)]),
        ]
        il_pos = 0

        def run_il_sched():
            nonlocal il_pos
            while il_pos < len(il_sched) and il_sched[il_pos][0] <= covered:
                for act in il_sched[il_pos][1]:
                    if act[0] == "a":
                        emit_a(act[1], *a_specs[act[1]])
                    else:
                        unlock_il(act[1], act[2])
                il_pos += 1

        emit_a("g0", *a_specs["g0"])
        unlock_il(0, 1)
        unlock_ready_groups()          # groups within xc (j0, j1)
        relu(XC, 512)
        covered = 4
        unlock_il(2, 3)
        unlock_ready_groups()          # j2, j3
        for c0, c1 in relu_split:
            relu(c0, c1)
            covered = c1 // 128
            run_il_sched()
            unlock_ready_groups()

    nc.compile()
    return nc


def _pack_kt(arr_t, nfree):
    """(D, nfree) k-major -> (128, KT*nfree) partition-packed SBUF layout."""
    return np.ascontiguousarray(
        arr_t.reshape(KT, 128, nfree).transpose(1, 0, 2).reshape(128, KT * nfree)
    )


def make_in_maps(x, W, bias):
    x = np.asarray(x, np.float16)
    W = np.asarray(W, np.float16)
    bias = np.asarray(bias, np.float16)
    x_flat = x.reshape(T, D)
    w_all = np.ascontiguousarray(
        np.ascontiguousarray(W.T)
        .reshape(2 * KT, 128, D)
        .transpose(1, 0, 2)
        .reshape(128, 2 * KT * D)
    )
    b2 = np.ascontiguousarray(bias.reshape(1, D))
    maps = []
    for r in range(NCORES):
        xr = np.roll(x_flat, -r * TI, axis=0)
        xTr = _pack_kt(np.ascontiguousarray(xr.T), T)
        maps.append({"xT": xTr, "w_in": w_all, "bias": b2})
    return maps


_NC_CACHE = {}


def get_nc(repeats=1, group_sizes=(1, 1, 2, 4, 4), warm=4, **kw):
    key = (repeats, tuple(group_sizes), warm, tuple(sorted(kw.items())))
    if key not in _NC_CACHE:
        _NC_CACHE[key] = build_nc(
            repeats=repeats, group_sizes=group_sizes, warm=warm, **kw
        )
    return _NC_CACHE[key]


def kernel(x, W, bias, group_sizes=(1, 1, 2, 4, 4), warm=4, **kw):
    nc = get_nc(1, group_sizes, warm, **kw)
    maps = make_in_maps(x, W, bias)
    res = run_bass_kernel_spmd(nc, maps, list(range(NCORES)))
    parts = []
    for r in range(NCORES):
        o = res.results[r]["out"].reshape(LPC, T, D)
        parts.append(np.roll(o, r * TI, axis=1))
    full = np.concatenate(parts, axis=0)          # (L, T, D)
    return full.reshape(L * L, Bdim, D).astype(np.float32)
